# revision 1
# baseline (speedup 1.0000x reference)
import numpy as np

# nn_AdderModel: B=16384, T=64, VOCAB=10, D=3, HD=4, FF=2. 8-core data parallel:
# shard batch 2048 rows/core. Host precomputes the (c=vocab,t)-indexed tables and
# the small per-token tensors; the device kernel computes the rank-2 -> VOCAB
# logits expansion and writes the full 16384x64x10 output (the memory-dominant
# stage) on 8 NeuronCores via bass/Tile.

B, T, VOCAB, D, HD, FF = 16384, 64, 10, 3, 4, 2
EPS = 1e-6
NCORES = 8
RPC = B // NCORES  # 2048 rows per core
G = RPC // 128     # 16 row-groups of 128 partitions


def _rms(x, w):
    return x / np.sqrt(np.mean(x * x, axis=-1, keepdims=True) + EPS) * w


def _rope(x, theta=3.0):
    t = np.arange(x.shape[-2], dtype=x.dtype)
    inv_freq = 1.0 / theta ** (np.arange(0, HD, 2, dtype=x.dtype) / HD)
    freqs = np.outer(t, inv_freq)
    cos_f, sin_f = np.cos(freqs), np.sin(freqs)
    x1, x2 = x[..., ::2], x[..., 1::2]
    rot = np.stack([x1 * cos_f - x2 * sin_f, x1 * sin_f + x2 * cos_f], axis=-1)
    return rot.reshape(x.shape)


def _host_forward(idx, arc_A, arc_start, arc_stride, w_ln1, w_ln2, w_lnf, w_qn,
                  Wq, Wk, Wg, Wu, Wd):
    """Everything up to the final [...,:2] @ table.T, in float64-free numpy f32."""
    f32 = np.float32
    digits = np.arange(VOCAB, dtype=f32)
    angles = arc_start + digits * arc_stride
    table = np.stack([arc_A * np.cos(angles), arc_A * np.sin(angles)], axis=1)

    tok = table[idx]                                            # [B,T,2]
    pe = np.sin(np.arange(T, dtype=f32) * np.exp(np.asarray(-np.log(10000.0), f32)))
    pos = np.broadcast_to(pe[None, :, None], (idx.shape[0], T, 1))
    x = np.concatenate([tok, pos], axis=-1).astype(f32)          # [B,T,3]

    h = _rms(x, w_ln1)
    q = _rms(h @ Wq.T, w_qn)
    k = _rms(h @ Wk.T, w_qn)
    v = h @ Wk.T
    q = _rope(q)
    k = _rope(k)

    scale = HD ** (-0.5)
    scores = np.einsum("btd,bsd->bts", q, k).astype(f32) * scale
    causal = np.triu(np.ones((T, T), dtype=bool), k=1)
    scores = np.where(causal, -np.inf, scores)
    scores -= scores.max(axis=-1, keepdims=True)
    e = np.exp(scores)
    attn = e / e.sum(axis=-1, keepdims=True)
    out = np.einsum("bts,bsd->btd", attn, v).astype(f32)

    x = x + out @ Wq
    h = _rms(x, w_ln2)
    x = x + (h @ Wg.T / (1 + np.exp(-(h @ Wg.T))) * (h @ Wu.T)) @ Wd.T
    x = _rms(x, w_lnf)
    return x[..., :2].astype(f32), table.astype(f32)             # [B,T,2], [10,2]


_NC_CACHE = {}


def _build_device_kernel(tab):
    """Bass kernel: per core, xf0/xf1 [2048,64] f32 -> out [2048, 64*10] f32,
    out[b, t*10+v] = xf0[b,t]*tab[v,0] + xf1[b,t]*tab[v,1]."""
    import concourse.bass as bass
    import concourse.mybir as mybir

    nc = bass.Bass()
    xf0 = nc.dram_tensor("xf0", (RPC, T), mybir.dt.float32, kind="ExternalInput")
    xf1 = nc.dram_tensor("xf1", (RPC, T), mybir.dt.float32, kind="ExternalInput")
    tabs = nc.dram_tensor("tabs", (128, 2 * VOCAB), mybir.dt.float32, kind="ExternalInput")
    out = nc.dram_tensor("out", (RPC, T * VOCAB), mybir.dt.float32, kind="ExternalOutput")

    GT = G * T  # 1024 free elems when all groups are packed on one partition row
    # partition p <-> DRAM rows [16p, 16p+16): contiguous per-partition transfers
    x0r = xf0.rearrange("(p g) t -> p (g t)", p=128)
    x1r = xf1.rearrange("(p g) t -> p (g t)", p=128)
    outr = out.rearrange("(p g) n -> p (g n)", p=128)

    with (
        nc.sbuf_tensor([128, 2 * VOCAB], mybir.dt.float32) as tt,
        nc.sbuf_tensor([128, GT], mybir.dt.float32) as a,
        nc.sbuf_tensor([128, GT], mybir.dt.float32) as b,
        nc.sbuf_tensor([128, GT * VOCAB], mybir.dt.float32) as o,
        nc.sbuf_tensor([128, GT * VOCAB], mybir.dt.float32) as w,
        nc.semaphore() as dsem,
        nc.semaphore() as vsem,
        nc.Block() as block,
    ):
        @block.sync
        def _(sync):
            sync.dma_start(tt[:, :], tabs[:, :]).then_inc(dsem, 16)
            sync.dma_start(a[:, :], x0r).then_inc(dsem, 16)
            sync.dma_start(b[:, :], x1r).then_inc(dsem, 16)
            sync.wait_ge(vsem, 3)
            sync.dma_start(outr, o[:, :]).then_inc(dsem, 16)

        @block.vector
        def _(vector):
            vector.wait_ge(dsem, 48)
            t0b = tt[:, 0:VOCAB][:, None, :].broadcast_to([128, GT, VOCAB])
            t1b = tt[:, VOCAB:2 * VOCAB][:, None, :].broadcast_to([128, GT, VOCAB])
            o3 = o[:, :].rearrange("p (t v) -> p t v", v=VOCAB)
            w3 = w[:, :].rearrange("p (t v) -> p t v", v=VOCAB)
            ab = a[:, :, None].broadcast_to([128, GT, VOCAB])
            bb = b[:, :, None].broadcast_to([128, GT, VOCAB])
            vector.tensor_mul(o3, ab, t0b).then_inc(vsem, 1)
            vector.tensor_mul(w3, bb, t1b).then_inc(vsem, 1)
            vector.tensor_add(o[:, :], o[:, :], w[:, :]).then_inc(vsem, 1)
    return nc


def kernel(**inputs):
    idx = np.asarray(inputs["idx"])
    args = {k: np.asarray(v, np.float32) for k, v in inputs.items() if k != "idx"}
    xf, table = _host_forward(idx.astype(np.int64), **args)
    xf0 = np.ascontiguousarray(xf[..., 0], dtype=np.float32)     # [B,T]
    xf1 = np.ascontiguousarray(xf[..., 1], dtype=np.float32)

    from concourse.bass_utils import run_bass_kernel_spmd

    key = tuple(np.round(table.reshape(-1), 6).tolist())
    if key not in _NC_CACHE:
        _NC_CACHE[key] = _build_device_kernel(table)
    nc = _NC_CACHE[key]

    tabs = np.ascontiguousarray(
        np.broadcast_to(table.T.reshape(1, -1), (128, 2 * VOCAB)), np.float32)
    in_maps = [
        {"xf0": xf0[c * RPC:(c + 1) * RPC], "xf1": xf1[c * RPC:(c + 1) * RPC],
         "tabs": tabs}
        for c in range(NCORES)
    ]
    res = run_bass_kernel_spmd(nc, in_maps, core_ids=list(range(NCORES)))
    outs = [res.results[c]["out"].reshape(RPC, T, VOCAB) for c in range(NCORES)]
    return np.concatenate(outs, axis=0)


if __name__ == "__main__":
    rng = np.random.default_rng(0)
    demo = {
        "idx": rng.integers(0, VOCAB, (B, T)).astype(np.int32),
        "arc_A": np.float32(2.5), "arc_start": np.float32(-1.2),
        "arc_stride": np.float32(0.29),
        "w_ln1": np.ones(D, np.float32), "w_ln2": np.ones(D, np.float32),
        "w_lnf": np.ones(D, np.float32), "w_qn": np.ones(HD, np.float32),
        "Wq": rng.standard_normal((HD, D)).astype(np.float32) * 0.5,
        "Wk": rng.standard_normal((HD, D)).astype(np.float32) * 0.5,
        "Wg": rng.standard_normal((FF, D)).astype(np.float32) * 0.5,
        "Wu": rng.standard_normal((FF, D)).astype(np.float32) * 0.5,
        "Wd": rng.standard_normal((D, FF)).astype(np.float32) * 0.5,
    }
    o = kernel(**demo)
    print("out", o.shape, o.dtype, float(np.abs(o).mean()))



# revision 11
# speedup vs baseline: 3125.6452x; 3125.6452x over previous
import math
import numpy as np
import ml_dtypes

# nn_AdderModel on 8 NeuronCores, data-parallel over batch (2048 rows/core).
#
# The whole idx-dependent forward runs ON DEVICE. Host only precomputes tiny
# parameter-derived tables (the "replicated parameter set"):
#   q/k/v per (position t, digit i) -> 640 combos; from those a masked score
#   table Etab[m, n] over m=(j,s) [source], n=(t,i) [target], with 4 planes:
#     plane 0:  den[n, m]   = exp(q_ti . k_sj / sqrt(HD)) * [s <= t]
#     plane 1+c: a_c[n, m]  = sum_d den*v_d * Wq[d, c]   (out-proj folded in)
# Device, per 128-row chunk:
#   C^T[m, b] one-hot of idx  -> TensorE: psum[b, n-planes] = sum_m C^T * Etab
#   select per (b, t): C[b, n] (x) planes, segment-reduce over i (vocab)
#   tail: y = x + a/den; rms (via Ln/Exp); silu MLP; final rms; logits expand.
# x comes from ACT Sin LUT directly on idx (embedding is a circular arc).

B, T, VOCAB, D, HD, FF = 16384, 64, 10, 3, 4, 2
EPS = 1e-6
NCORES = 8
RPC = B // NCORES          # 2048 rows per core
NCHUNK = RPC // 128        # 16 chunks of 128 partitions
NM = T * VOCAB             # 640 = contraction size (m) and per-plane n size
NPL = 1 + D                # planes: den, a0, a1, a2
NCOL = NPL * NM            # 2560 psum columns
KT = NM // 128             # 5 m-tiles

_f32 = np.float32
_bf16 = ml_dtypes.bfloat16


def _rms_np(x, w):
    return x / np.sqrt(np.mean(x * x, axis=-1, keepdims=True) + EPS) * w


def _rope_np(x, theta=3.0):
    t = np.arange(x.shape[-2], dtype=x.dtype)
    inv_freq = 1.0 / theta ** (np.arange(0, HD, 2, dtype=x.dtype) / HD)
    freqs = np.outer(t, inv_freq)
    cos_f, sin_f = np.cos(freqs), np.sin(freqs)
    x1, x2 = x[..., ::2], x[..., 1::2]
    rot = np.stack([x1 * cos_f - x2 * sin_f, x1 * sin_f + x2 * cos_f], axis=-1)
    return rot.reshape(x.shape)


def _host_tables(arc_A, arc_start, arc_stride, w_ln1, w_ln2, w_lnf, w_qn,
                 Wq, Wk, Wg, Wu, Wd):
    """Parameter-derived constant tables (no idx dependence)."""
    digits = np.arange(VOCAB, dtype=_f32)
    angles = arc_start + digits * arc_stride
    table = np.stack([arc_A * np.cos(angles), arc_A * np.sin(angles)], axis=1)
    pe = np.sin(np.arange(T, dtype=_f32) * np.exp(np.asarray(-np.log(10000.0), _f32)))

    Xtab = np.zeros((T, VOCAB, D), _f32)
    Xtab[:, :, 0] = table[None, :, 0]
    Xtab[:, :, 1] = table[None, :, 1]
    Xtab[:, :, 2] = pe[:, None]

    h = _rms_np(Xtab, w_ln1)
    q = _rms_np(h @ Wq.T, w_qn)
    k = _rms_np(h @ Wk.T, w_qn)
    v = h @ Wk.T
    q = _rope_np(q.transpose(1, 0, 2)).transpose(1, 0, 2)   # rope along t
    k = _rope_np(k.transpose(1, 0, 2)).transpose(1, 0, 2)

    sc = np.einsum("tid,sjd->tisj", q, k) * (HD ** -0.5)    # [T,10,T,10]
    mask = (np.arange(T)[:, None, None, None] >= np.arange(T)[None, None, :, None])
    E = (np.exp(sc) * mask).astype(_f32)                    # den plane
    Atab = np.einsum("tisj,sjd,dc->tisjc", E, v, Wq).astype(_f32)

    # etab[m, col]: m = j*64 + s (j-major), col = plane*640 + t*10 + i
    etab = np.zeros((NM, NCOL), _f32)
    E_m = E.transpose(3, 2, 0, 1).reshape(NM, NM)           # [(j,s), (t,i)]
    etab[:, 0:NM] = E_m
    A_m = Atab.transpose(3, 2, 0, 1, 4).reshape(NM, NM, D)
    for c in range(D):
        etab[:, (1 + c) * NM:(2 + c) * NM] = A_m[:, :, c]

    # jtab[p, k] = 2k + p//64  (digit id for C^T partition p of m-tile k)
    jtab = np.zeros((128, 8), np.int32)
    for kk in range(KT):
        jtab[:64, kk] = 2 * kk
        jtab[64:, kk] = 2 * kk + 1

    pe_rep = np.broadcast_to(pe[None, :], (128, T)).copy()

    # ttab[p, v] = w_lnf[0]*table[v,0]; ttab[p, 10+v] = w_lnf[1]*table[v,1]
    ttab = np.zeros((128, 2 * VOCAB), _f32)
    ttab[:, :VOCAB] = w_lnf[0] * table[:, 0]
    ttab[:, VOCAB:] = w_lnf[1] * table[:, 1]

    Wgp = (Wg * w_ln2[None, :]).astype(_f32)   # fold w_ln2 into MLP weights
    Wup = (Wu * w_ln2[None, :]).astype(_f32)
    consts = dict(
        A=float(arc_A), start=float(arc_start), stride=float(arc_stride),
        Wgp=Wgp, Wup=Wup, Wd=np.asarray(Wd, _f32),
    )
    return (etab.astype(_bf16), jtab, pe_rep.astype(_f32),
            ttab.astype(_bf16), consts)


def _build_nc(consts, reps=1):
    import contextlib
    import concourse.bass as bass
    import concourse.bacc as bacc
    import concourse.mybir as mybir
    import concourse.tile as tile

    fp32 = mybir.dt.float32
    bf16 = mybir.dt.bfloat16
    i32 = mybir.dt.int32
    AF = mybir.ActivationFunctionType
    OP = mybir.AluOpType
    AX = mybir.AxisListType

    A = consts["A"]; start = consts["start"]; stride = consts["stride"]
    Wgp = consts["Wgp"]; Wup = consts["Wup"]; Wd = consts["Wd"]

    nc = bacc.Bacc()
    idx_d = nc.dram_tensor("idx", (RPC, T), i32, kind="ExternalInput")
    idxt_d = nc.dram_tensor("idxt", (128, RPC), i32, kind="ExternalInput")
    jtab_d = nc.dram_tensor("jtab", (128, 8), i32, kind="ExternalInput")
    etab_d = nc.dram_tensor("etab", (NM, NCOL), bf16, kind="ExternalInput")
    pe_d = nc.dram_tensor("pe", (128, T), fp32, kind="ExternalInput")
    ttab_d = nc.dram_tensor("ttab", (128, 2 * VOCAB), bf16, kind="ExternalInput")
    out_d = nc.dram_tensor("out", (RPC, T * VOCAB), fp32, kind="ExternalOutput")

    with tile.TileContext(nc) as tc:
        rep_ctx = tc.For_i(0, reps) if reps > 1 else contextlib.nullcontext()
        with rep_ctx, tc.tile_pool(name="persist", bufs=1) as pp_pool:
            # ---- persistent tiles (live through tail) ----
            acc4 = pp_pool.tile([128, NPL, NCHUNK * T], fp32)   # den,a0..a2
            x01 = pp_pool.tile([128, 2, NCHUNK * T], fp32)      # tok embeddings
            pe_s = pp_pool.tile([128, T], fp32)
            ttab_s = pp_pool.tile([128, 2 * VOCAB], bf16)
            nc.sync.dma_start(pe_s[:], pe_d[:])
            nc.sync.dma_start(ttab_s[:], ttab_d[:])
            cst = pp_pool.tile([128, 4], fp32)   # activation bias constants
            nc.gpsimd.memset(cst[:, 0:1], start + math.pi / 2)
            nc.gpsimd.memset(cst[:, 1:2], start)
            nc.gpsimd.memset(cst[:, 2:3], EPS)
            b_cos, b_sin, b_eps = cst[:, 0:1], cst[:, 1:2], cst[:, 2:3]

            with (
                tc.tile_pool(name="phase1", bufs=1) as p1,
                tc.tile_pool(name="work", bufs=3) as wk,
                tc.tile_pool(name="psum", bufs=1, space="PSUM") as ps,
            ):
                # ---- phase-1 constants ----
                etab_s = p1.tile([128, KT, NCOL], bf16)
                nc.sync.dma_start(
                    etab_s[:], etab_d.rearrange("(k p) n -> p k n", p=128))
                jtab_s = p1.tile([128, 8], i32)
                nc.sync.dma_start(jtab_s[:], jtab_d[:])
                idxt2 = p1.tile([128, RPC], i32)
                nc.sync.dma_start(idxt2[:], idxt_d[:])
                iota_t = p1.tile([128, NM], i32)
                nc.gpsimd.iota(iota_t[:], pattern=[[0, T], [1, VOCAB]],
                               base=0, channel_multiplier=0)
                # C^T[m=(j,s), b]: ct[p, k, b] = (idxt2[p,b] == jtab[p,k])
                ct = p1.tile([128, KT, RPC], bf16)
                for k in range(KT):
                    nc.vector.tensor_tensor(
                        ct[:, k, :], idxt2[:],
                        jtab_s[:, k:k + 1].broadcast_to([128, RPC]),
                        op=OP.is_equal)

                for c in range(NCHUNK):
                    idx_s = wk.tile([128, T], i32, tag="idx")
                    nc.sync.dma_start(idx_s[:], idx_d[c * 128:(c + 1) * 128, :])
                    # one-hot C[b, (t,i)]
                    cb = wk.tile([128, NM], bf16, tag="cb")
                    nc.vector.tensor_tensor(
                        cb[:].rearrange("p (t i) -> p t i", i=VOCAB),
                        idx_s[:, :, None].broadcast_to([128, T, VOCAB]),
                        iota_t[:].rearrange("p (t i) -> p t i", i=VOCAB),
                        op=OP.is_equal)
                    # token embedding via arc: x0=A*cos(th), x1=A*sin(th)
                    idxf = wk.tile([128, T], fp32, tag="idxf")
                    nc.scalar.copy(idxf[:], idx_s[:])
                    tr0 = wk.tile([128, T], fp32, tag="tr0")
                    nc.scalar.activation(tr0[:], idxf[:], AF.Sin,
                                         bias=b_cos, scale=stride)
                    nc.vector.tensor_scalar_mul(x01[:, 0, c * T:(c + 1) * T],
                                                tr0[:], A)
                    tr1 = wk.tile([128, T], fp32, tag="tr1")
                    nc.scalar.activation(tr1[:], idxf[:], AF.Sin,
                                         bias=b_sin, scale=stride)
                    nc.vector.tensor_scalar_mul(x01[:, 1, c * T:(c + 1) * T],
                                                tr1[:], A)

                    # pass-1 matmuls: psum[b, col] = sum_m C^T[m,b] * etab[m,col]
                    pmm = ps.tile([128, NCOL], fp32, tag="pmm")
                    pl_bf = wk.tile([128, NCOL], bf16, tag="plbf")
                    for nb in range(NCOL // 512):
                        for k in range(KT):
                            nc.tensor.matmul(
                                pmm[:, nb * 512:(nb + 1) * 512],
                                ct[:, k, c * 128:(c + 1) * 128],
                                etab_s[:, k, nb * 512:(nb + 1) * 512],
                                start=(k == 0), stop=(k == KT - 1))
                        nc.scalar.copy(pl_bf[:, nb * 512:(nb + 1) * 512],
                                       pmm[:, nb * 512:(nb + 1) * 512])
                    # select own-digit target: multiply by C, reduce over i
                    sel = wk.tile([128, NCOL], bf16, tag="sel")
                    nc.vector.tensor_mul(
                        sel[:].rearrange("p (pl n) -> p pl n", pl=NPL),
                        pl_bf[:].rearrange("p (pl n) -> p pl n", pl=NPL),
                        cb[:, None, :].broadcast_to([128, NPL, NM]))
                    nc.vector.tensor_reduce(
                        acc4[:, :, c * T:(c + 1) * T],
                        sel[:].rearrange("p (pl t i) -> p pl t i", pl=NPL,
                                         i=VOCAB),
                        axis=AX.X, op=OP.add)

            # ================= tail =================
            NT = NCHUNK * T  # 1024
            with tc.tile_pool(name="tail", bufs=1) as tl:
                den = acc4[:, 0, :]
                r = tl.tile([128, NT], fp32)
                nc.scalar.activation(r[:], den, AF.Ln)
                nc.scalar.activation(r[:], r[:], AF.Exp, scale=-1.0)

                y = tl.tile([128, D, NT], fp32)
                for cc in range(D):
                    nc.vector.tensor_mul(y[:, cc, :], acc4[:, 1 + cc, :], r[:])
                nc.vector.tensor_add(y[:, 0, :], y[:, 0, :], x01[:, 0, :])
                nc.vector.tensor_add(y[:, 1, :], y[:, 1, :], x01[:, 1, :])
                nc.vector.tensor_add(
                    y[:, 2, :].rearrange("p (c t) -> p c t", t=T),
                    y[:, 2, :].rearrange("p (c t) -> p c t", t=T),
                    pe_s[:, None, :].broadcast_to([128, NCHUNK, T]))

                tmp = tl.tile([128, NT], fp32)
                ss = tl.tile([128, NT], fp32)
                inv = tl.tile([128, NT], fp32)

                def rms_inv(src3):
                    nc.scalar.activation(ss[:], src3[:, 0, :], AF.Square)
                    nc.scalar.activation(tmp[:], src3[:, 1, :], AF.Square)
                    nc.vector.tensor_add(ss[:], ss[:], tmp[:])
                    nc.scalar.activation(tmp[:], src3[:, 2, :], AF.Square)
                    nc.vector.tensor_add(ss[:], ss[:], tmp[:])
                    nc.scalar.activation(inv[:], ss[:], AF.Ln, bias=b_eps,
                                         scale=1.0 / D)
                    nc.scalar.activation(inv[:], inv[:], AF.Exp, scale=-0.5)

                rms_inv(y)
                h = tl.tile([128, D, NT], fp32)
                for cc in range(D):
                    nc.vector.tensor_mul(h[:, cc, :], y[:, cc, :], inv[:])

                # MLP: g/u = h @ Wgp.T / Wup.T  (FF=2)
                gu = tl.tile([128, 2 * FF, NT], fp32, tag="guy2")
                for fi, W in ((0, Wgp), (1, Wup)):
                    for f in range(FF):
                        o = gu[:, fi * FF + f, :]
                        nc.vector.tensor_scalar_mul(tmp[:], h[:, 2, :],
                                                    float(W[f, 2]))
                        nc.vector.scalar_tensor_tensor(
                            o, h[:, 1, :], float(W[f, 1]), tmp[:],
                            op0=OP.mult, op1=OP.add)
                        nc.vector.scalar_tensor_tensor(
                            o, h[:, 0, :], float(W[f, 0]), o,
                            op0=OP.mult, op1=OP.add)
                pr = tl.tile([128, FF, NT], fp32)
                for f in range(FF):
                    nc.scalar.activation(tmp[:], gu[:, f, :], AF.Sigmoid)
                    nc.vector.tensor_mul(tmp[:], tmp[:], gu[:, f, :])
                    nc.vector.tensor_mul(pr[:, f, :], tmp[:], gu[:, FF + f, :])
                # y2 = y + pr @ Wd.T (reuses the gu slot; disjoint lifetime)
                y2 = tl.tile([128, D, NT], fp32, tag="guy2")
                for cc in range(D):
                    nc.vector.tensor_scalar_mul(tmp[:], pr[:, 0, :],
                                                float(Wd[cc, 0]))
                    nc.vector.scalar_tensor_tensor(
                        tmp[:], pr[:, 1, :], float(Wd[cc, 1]), tmp[:],
                        op0=OP.mult, op1=OP.add)
                    nc.vector.tensor_add(y2[:, cc, :], y[:, cc, :], tmp[:])
                rms_inv(y2)
                z = tl.tile([128, 2, NT], bf16)
                nc.vector.tensor_mul(z[:, 0, :], y2[:, 0, :], inv[:])
                nc.vector.tensor_mul(z[:, 1, :], y2[:, 1, :], inv[:])

                # logits in half-passes to bound temp size
                lg = tl.tile([128, NT * VOCAB], fp32)
                HNT = NT // 2
                for hh in range(2):
                    lgA = tl.tile([128, HNT, VOCAB], bf16, tag="lgA")
                    nc.vector.tensor_mul(
                        lgA[:],
                        z[:, 0, hh * HNT:(hh + 1) * HNT, None].broadcast_to(
                            [128, HNT, VOCAB]),
                        ttab_s[:, None, 0:VOCAB].broadcast_to(
                            [128, HNT, VOCAB]))
                    lgB = tl.tile([128, HNT, VOCAB], bf16, tag="lgB")
                    nc.vector.tensor_mul(
                        lgB[:],
                        z[:, 1, hh * HNT:(hh + 1) * HNT, None].broadcast_to(
                            [128, HNT, VOCAB]),
                        ttab_s[:, None, VOCAB:].broadcast_to(
                            [128, HNT, VOCAB]))
                    nc.vector.tensor_add(
                        lg[:, hh * HNT * VOCAB:(hh + 1) * HNT * VOCAB]
                        .rearrange("p (t v) -> p t v", v=VOCAB),
                        lgA[:], lgB[:])
                nc.sync.dma_start(
                    out_d.rearrange("(c p) n -> p c n", p=128),
                    lg[:].rearrange("p (c n) -> p c n", c=NCHUNK))
    nc.finalize()
    return nc


_NC_CACHE = {}


def _get_nc(key, consts, reps=1):
    if (key, reps) not in _NC_CACHE:
        _NC_CACHE[(key, reps)] = _build_nc(consts, reps)
    return _NC_CACHE[(key, reps)]


def _prep(inputs):
    idx = np.ascontiguousarray(np.asarray(inputs["idx"], np.int32))
    pnames = ["arc_A", "arc_start", "arc_stride", "w_ln1", "w_ln2", "w_lnf",
              "w_qn", "Wq", "Wk", "Wg", "Wu", "Wd"]
    params = [np.asarray(inputs[p], _f32) for p in pnames]
    etab, jtab, pe_rep, ttab, consts = _host_tables(*params)
    key = hash(tuple(np.asarray(p, _f32).tobytes() for p in params))
    in_maps = []
    for c in range(NCORES):
        ic = idx[c * RPC:(c + 1) * RPC]
        in_maps.append({
            "idx": ic,
            "idxt": np.ascontiguousarray(np.concatenate([ic.T, ic.T], axis=0)),
            "jtab": jtab, "etab": etab, "pe": pe_rep, "ttab": ttab,
        })
    return key, consts, in_maps


def kernel(**inputs):
    from concourse.bass_utils import run_bass_kernel_spmd
    key, consts, in_maps = _prep(inputs)
    nc = _get_nc(key, consts)
    res = run_bass_kernel_spmd(nc, in_maps, core_ids=list(range(NCORES)))
    outs = [res.results[c]["out"].reshape(RPC, T, VOCAB) for c in range(NCORES)]
    return np.concatenate(outs, axis=0).astype(np.float32)


if __name__ == "__main__":
    rng = np.random.default_rng(0)
    demo = {
        "idx": rng.integers(0, VOCAB, (B, T)).astype(np.int32),
        "arc_A": np.float32(2.5), "arc_start": np.float32(-1.2),
        "arc_stride": np.float32(0.29),
        "w_ln1": np.ones(D, np.float32), "w_ln2": np.ones(D, np.float32),
        "w_lnf": np.ones(D, np.float32), "w_qn": np.ones(HD, np.float32),
        "Wq": rng.standard_normal((HD, D)).astype(np.float32) * 0.5,
        "Wk": rng.standard_normal((HD, D)).astype(np.float32) * 0.5,
        "Wg": rng.standard_normal((FF, D)).astype(np.float32) * 0.5,
        "Wu": rng.standard_normal((FF, D)).astype(np.float32) * 0.5,
        "Wd": rng.standard_normal((D, FF)).astype(np.float32) * 0.5,
    }
    o = kernel(**demo)
    print("out", o.shape, o.dtype, float(np.abs(o).mean()))
